# revision 11
# baseline (speedup 1.0000x reference)
import sys

import numpy as np

for _p in ("/opt/trn_rl_repo",):
    if _p not in sys.path:
        sys.path.insert(0, _p)

import concourse.bass as bass
import concourse.bacc as bacc
import concourse.mybir as mybir
import concourse.tile as tile
from concourse.bass_utils import run_bass_kernel_spmd

F32 = mybir.dt.float32
AF = mybir.ActivationFunctionType

# Problem dims (hardcoded per contract)
B, S, E, H = 2, 2048, 512, 32
D = E // H            # 16
NCORE = 8
HPC = H // NCORE      # 4 heads per core
C = HPC * D           # 64 channels per core
JCOLS = 2432          # shifted-weight row length: lag = j - 384 - p in [0, 2047]
PAD = -200.0          # softplus(PAD) == 0.0 in fp32
NT = S // 512         # 4 t-blocks of 512
NST = S // 128        # 16 s-tiles of 128

_CACHE = {}


def _build_program():
    nc = bacc.Bacc()

    xT_d = nc.dram_tensor("xT", [128, B * S // 128, E], F32, kind="ExternalInput")
    inpwT_d = nc.dram_tensor("inpwT", [E, C], F32, kind="ExternalInput")
    inpb_d = nc.dram_tensor("inpb", [128, C], F32, kind="ExternalInput")
    wsh_d = nc.dram_tensor("wsh", [HPC, 128, JCOLS], F32, kind="ExternalInput")
    wrawT_d = nc.dram_tensor("wrawT", [128, C], F32, kind="ExternalInput")
    bias_d = nc.dram_tensor("bias", [HPC, S], F32, kind="ExternalInput")
    womask_d = nc.dram_tensor("womask", [B, 128, E], F32, kind="ExternalInput")
    lts_d = nc.dram_tensor("lts", [128, 128], F32, kind="ExternalInput")
    ones_d = nc.dram_tensor("ones32", [1, 32], F32, kind="ExternalInput")
    out_d = nc.dram_tensor("out", [B, S, E], F32, kind="ExternalOutput")

    with tile.TileContext(nc) as tc:
        with (
            tc.tile_pool(name="persist", bufs=1) as pp,
            tc.tile_pool(name="wstage", bufs=2) as wstage,
            tc.tile_pool(name="psh", bufs=2, space="PSUM") as psh,
            tc.tile_pool(name="psy", bufs=2, space="PSUM") as psy,
            tc.tile_pool(name="pso", bufs=2, space="PSUM") as pso,
            tc.tile_pool(name="pse", bufs=1, space="PSUM") as pse,
        ):
            # ---- small/constant loads ----
            wt = pp.tile([128, 4, C], F32, tag="wt")  # inp_wT as (p, k, c)
            nc.sync.dma_start(
                wt[:], inpwT_d[:].rearrange("(k p) c -> p k c", p=128)
            )
            inpb = pp.tile([128, C], F32, tag="inpb")
            nc.sync.dma_start(inpb[:], inpb_d[:])
            wrawT = pp.tile([128, C], F32, tag="wrawT")
            nc.sync.dma_start(wrawT[:], wrawT_d[:])
            biasr = pp.tile([HPC, S], F32, tag="biasr")
            nc.sync.dma_start(biasr[:], bias_d[:])
            lts = pp.tile([128, 128], F32, tag="lts")
            nc.sync.dma_start(lts[:], lts_d[:])
            ones32 = pp.tile([1, 32], F32, tag="ones32")
            nc.sync.dma_start(ones32[:], ones_d[:])
            wom = []
            for b in range(B):
                t = pp.tile([128, E], F32, tag=f"wom{b}")
                nc.sync.dma_start(t[:], womask_d[b])
                wom.append(t)

            # ---- norm = cumsum(softplus(w)) ; layout s = 16p + f ----
            wsp = pp.tile([128, C], F32, tag="wsp")
            nc.scalar.activation(wsp[:], wrawT[:], AF.Exp)
            nc.scalar.activation(wsp[:], wsp[:], AF.Ln, bias=1.0)
            cum = [wsp]
            for i, k in enumerate((1, 2, 4, 8)):
                nxt = pp.tile([128, C], F32, tag=f"cum{i}")
                prev = cum[-1]
                pv = prev[:].rearrange("p (g f) -> p g f", g=HPC)
                nv = nxt[:].rearrange("p (g f) -> p g f", g=HPC)
                nc.vector.tensor_add(
                    nv[:, :, k:16], pv[:, :, k:16], pv[:, :, 0 : 16 - k]
                )
                nc.vector.tensor_copy(nv[:, :, 0:k], pv[:, :, 0:k])
                cum.append(nxt)
            cfin = cum[-1]
            # chunk totals (p, g) at f=15, exclusive prefix over partitions via PE
            t128 = cfin[:].rearrange("p (g f) -> p g f", g=HPC)[:, :, 15]
            pe_e = pse.tile([128, HPC], F32)
            nc.tensor.matmul(pe_e[:], lts[:], t128, start=True, stop=True)
            norm = pp.tile([128, C], F32, tag="norm")
            nc.vector.tensor_add(
                norm[:].rearrange("p (g f) -> p g f", g=HPC),
                cfin[:].rearrange("p (g f) -> p g f", g=HPC),
                pe_e[:, :, None].broadcast_to([128, HPC, 16]),
            )
            rnorm = pp.tile([128, C], F32, tag="rnorm")
            nc.vector.reciprocal(rnorm[:], norm[:])
            # rows layout (g, s) with s = 16p + f
            rnr = pp.tile([HPC, S], F32, tag="rnr")
            nrr = pp.tile([HPC, S], F32, tag="nrr")
            for g in range(HPC):
                nc.sync.dma_start(
                    rnr[g : g + 1, :], rnorm[:, g * 16 : (g + 1) * 16]
                )
                nc.sync.dma_start(
                    nrr[g : g + 1, :], norm[:, g * 16 : (g + 1) * 16]
                )
            bnorm = pp.tile([HPC, S], F32, tag="bnorm")
            nc.vector.tensor_mul(bnorm[:], biasr[:], nrr[:])
            bnr = pp.tile([1, HPC * S], F32, tag="bnr")
            for g in range(HPC):
                nc.sync.dma_start(
                    bnr[0:1, g * S : (g + 1) * S], bnorm[g : g + 1, :]
                )
            rmask = pp.tile([128, S], F32, tag="rmask")
            for g in range(HPC):
                for r in range(32):
                    nc.sync.dma_start(
                        rmask[g * 32 + r : g * 32 + r + 1, :], rnr[g : g + 1, :]
                    )

            # ---- shifted Toeplitz rows: softplus per head ----
            mstar = []
            for g in range(HPC):
                mt = pp.tile([128, JCOLS], F32, tag=f"mstar{g}", name=f"mstar{g}")
                nc.sync.dma_start(mt[:], wsh_d[g])
                nc.scalar.activation(mt[:], mt[:], AF.Exp)
                nc.scalar.activation(mt[:], mt[:], AF.Ln, bias=1.0)
                mstar.append(mt)

            # ---- input projection: psum_h[(s),(c)] per (st, b) ----
            h_sb = [pp.tile([128, 128], F32, tag=f"h{i}", name=f"h{i}") for i in range(NST)]
            for st in range(NST):
                for b in range(B):
                    m = b * NST + st
                    xm = wstage.tile([128, E], F32, tag="xm", bufs=6)
                    nc.sync.dma_start(xm[:], xT_d[:, m, :])
                    ph = psh.tile([128, C], F32)
                    for k in range(4):
                        nc.tensor.matmul(
                            ph[:],
                            xm[:, k * 128 : (k + 1) * 128],
                            wt[:, k, :],
                            start=(k == 0),
                            stop=(k == 3),
                        )
                    # scatter (g,d) -> columns 32g+16b+d, add inp_b
                    hv = h_sb[st][:].rearrange(
                        "p (g two d) -> p g two d", g=HPC, two=2
                    )[:, :, b, :]
                    nc.vector.tensor_add(
                        hv,
                        ph[:].rearrange("p (g d) -> p g d", g=HPC),
                        inpb[:].rearrange("p (g d) -> p g d", g=HPC),
                    )

            # ---- mixing: col-tiled 4 heads, accumulate over s-tiles ----
            y_sb = [pp.tile([128, 512], F32, tag=f"y{j}", name=f"y{j}") for j in range(NT)]
            for tj in range(NT):
                py = psy.tile([128, 512], F32)
                nsi = 4 * tj + 4
                for si in range(nsi):
                    j0 = 512 * tj - 128 * si + 384
                    for g in range(HPC):
                        nc.tensor.matmul(
                            py[32 * g : 32 * g + 32, :],
                            h_sb[si][:, 32 * g : 32 * g + 32],
                            mstar[g][:, j0 : j0 + 512],
                            start=(si == 0),
                            stop=False,
                            tile_position=(0, 32 * g),
                            skip_group_check=True,
                        )
                # + bias*norm as rank-1 (K=1) update
                for g in range(HPC):
                    nc.tensor.matmul(
                        py[32 * g : 32 * g + 32, :],
                        ones32[:],
                        bnr[0:1, g * S + 512 * tj : g * S + 512 * (tj + 1)],
                        start=False,
                        stop=True,
                        tile_position=(0, 32 * g),
                        skip_group_check=True,
                    )
                # y = psum * rnorm  (per head, column-wise multiplier)
                nc.vector.tensor_mul(
                    y_sb[tj][:],
                    py[:],
                    rmask[:, 512 * tj : 512 * (tj + 1)],
                )

            # ---- output projection (masked per b), DMA psum -> DRAM ----
            for b in range(B):
                for tt in range(NST):
                    tj, cc = tt // 4, (tt % 4) * 128
                    po = pso.tile([128, E], F32)
                    nc.tensor.matmul(
                        po[:],
                        y_sb[tj][:, cc : cc + 128],
                        wom[b][:],
                        start=True,
                        stop=True,
                    )
                    ob = wstage.tile([128, E], F32, tag="osb", bufs=3)
                    nc.any.tensor_copy(ob[:], po[:])
                    nc.sync.dma_start(
                        out_d[b, tt * 128 : (tt + 1) * 128, :], ob[:]
                    )
    nc.compile()
    return nc


def _host_prep(x, weight_raw, bias, inp_w, inp_b, out_w):
    x = np.asarray(x, np.float32)
    weight_raw = np.asarray(weight_raw, np.float32)
    bias = np.asarray(bias, np.float32)
    inp_w = np.asarray(inp_w, np.float32)
    inp_b = np.asarray(inp_b, np.float32)
    out_w = np.asarray(out_w, np.float32)

    xT = x.transpose(2, 0, 1).reshape(E, B * S)
    xT = np.ascontiguousarray(
        xT.reshape(4, 128, B * S // 128, 128).transpose(1, 2, 0, 3).reshape(
            128, B * S // 128, E
        )
    )

    p = np.arange(128)[:, None]
    j = np.arange(JCOLS)[None, :]
    lag = j - 384 - p
    valid = (lag >= 0) & (lag < S)
    lagc = np.clip(lag, 0, S - 1)

    lts = np.tril(np.ones((128, 128), np.float32), -1).T.copy()
    # lts[q, p] = 1 if q < p  -> strictly upper in (q, p) = tril(-1).T
    ones32 = np.ones((1, 32), np.float32)

    in_maps = []
    for core in range(NCORE):
        c0 = core * C
        heads = slice(core * HPC, (core + 1) * HPC)
        wsh = np.where(
            valid[None], weight_raw[heads][:, lagc], np.float32(PAD)
        ).astype(np.float32)
        wrawT = np.ascontiguousarray(
            weight_raw[heads].reshape(HPC, 128, 16).transpose(1, 0, 2).reshape(128, C)
        )
        # wrawT[p, 16g+f] = weight_raw[head g][16p+f]
        womask = np.zeros((B, 128, E), np.float32)
        wo_slice = out_w[:, c0 : c0 + C].T.astype(np.float32)  # (C=g*16+d, E)
        for b in range(B):
            v = womask[b].reshape(HPC, 2, D, E)
            v[:, b, :, :] = wo_slice.reshape(HPC, D, E)
        in_maps.append(
            {
                "xT": xT,
                "inpwT": np.ascontiguousarray(inp_w[c0 : c0 + C, :].T),
                "inpb": np.ascontiguousarray(np.broadcast_to(inp_b[c0 : c0 + C], (128, C))),
                "wsh": wsh,
                "wrawT": wrawT,
                "bias": bias[heads, :S].copy(),
                "womask": womask,
                "lts": lts,
                "ones32": ones32,
            }
        )
    return in_maps


def _run(in_maps, trace=False):
    if "nc" not in _CACHE:
        _CACHE["nc"] = _build_program()
    try:
        res = run_bass_kernel_spmd(
            _CACHE["nc"], in_maps, core_ids=list(range(NCORE)), trace=trace
        )
    except ModuleNotFoundError:
        res = run_bass_kernel_spmd(
            _CACHE["nc"], in_maps, core_ids=list(range(NCORE)), trace=False
        )
    return res


def kernel(x, weight_raw, bias, inp_w, inp_b, out_w, parallel=True, _trace=False):
    in_maps = _host_prep(x, weight_raw, bias, inp_w, inp_b, out_w)
    res = _run(in_maps, trace=_trace)
    out = np.zeros((B, S, E), np.float32)
    for r in res.results:
        out += r["out"]
    if _trace:
        kernel.last_exec_ns = res.exec_time_ns
        kernel.last_results = res
    return out


if __name__ == "__main__":
    rng = np.random.default_rng(0)
    inputs = {
        "x": rng.standard_normal((B, S, E), dtype=np.float32),
        "weight_raw": rng.standard_normal((H, S), dtype=np.float32),
        "bias": np.zeros((H, S), np.float32),
        "inp_w": rng.standard_normal((E, E), dtype=np.float32) / np.sqrt(E),
        "inp_b": np.zeros((E,), np.float32),
        "out_w": rng.standard_normal((E, E), dtype=np.float32) / np.sqrt(E),
    }
    o = kernel(**inputs)
    print("ok", o.shape, float(np.abs(o).mean()))


# revision 13
# speedup vs baseline: 1.2918x; 1.2918x over previous
import sys

import numpy as np

for _p in ("/opt/trn_rl_repo",):
    if _p not in sys.path:
        sys.path.insert(0, _p)

import concourse.bass as bass
import concourse.bacc as bacc
import concourse.mybir as mybir
import concourse.tile as tile
from concourse.bass_utils import run_bass_kernel_spmd

F32 = mybir.dt.float32
AF = mybir.ActivationFunctionType

# Problem dims (hardcoded per contract)
B, S, E, H = 2, 2048, 512, 32
D = E // H            # 16
NCORE = 8
HPC = H // NCORE      # 4 heads per core
C = HPC * D           # 64 channels per core
JCOLS = 2432          # shifted-weight row length: lag = j - 384 - p in [0, 2047]
PAD = -200.0          # softplus(PAD) == 0.0 in fp32
NT = S // 512         # 4 t-blocks of 512
NST = S // 128        # 16 s-tiles of 128

_CACHE = {}


def _build_program():
    nc = bacc.Bacc()

    xT_d = nc.dram_tensor("xT", [128, B * S // 128, E], F32, kind="ExternalInput")
    inpwT_d = nc.dram_tensor("inpwT", [E, C], F32, kind="ExternalInput")
    inpb_d = nc.dram_tensor("inpb", [128, C], F32, kind="ExternalInput")
    wsh_d = nc.dram_tensor("wsh", [HPC, 128, JCOLS], F32, kind="ExternalInput")
    wrawT_d = nc.dram_tensor("wrawT", [128, C], F32, kind="ExternalInput")
    bias_d = nc.dram_tensor("bias", [HPC, S], F32, kind="ExternalInput")
    womask_d = nc.dram_tensor("womask", [B, 128, E], F32, kind="ExternalInput")
    lts_d = nc.dram_tensor("lts", [128, 128], F32, kind="ExternalInput")
    sel4_d = nc.dram_tensor("sel4", [HPC, 128], F32, kind="ExternalInput")
    out_d = nc.dram_tensor("out", [B, S, E], F32, kind="ExternalOutput")

    with tile.TileContext(nc) as tc:
        with (
            tc.tile_pool(name="persist", bufs=1) as pp,
            tc.tile_pool(name="wstage", bufs=2) as wstage,
            tc.tile_pool(name="psh", bufs=2, space="PSUM") as psh,
            tc.tile_pool(name="psy", bufs=2, space="PSUM") as psy,
            tc.tile_pool(name="pso", bufs=2, space="PSUM") as pso,
            tc.tile_pool(name="pse", bufs=1, space="PSUM") as pse,
        ):
            # ---- small/constant loads ----
            wt = pp.tile([128, 4, C], F32, tag="wt")  # inp_wT as (p, k, c)
            nc.sync.dma_start(
                wt[:], inpwT_d[:].rearrange("(k p) c -> p k c", p=128)
            )
            inpb = pp.tile([128, C], F32, tag="inpb")
            nc.sync.dma_start(inpb[:], inpb_d[:])
            wrawT = pp.tile([128, C], F32, tag="wrawT")
            nc.sync.dma_start(wrawT[:], wrawT_d[:])
            biasr = pp.tile([HPC, S], F32, tag="biasr")
            nc.sync.dma_start(biasr[:], bias_d[:])
            lts = pp.tile([128, 128], F32, tag="lts")
            nc.sync.dma_start(lts[:], lts_d[:])
            sel4 = pp.tile([HPC, 128], F32, tag="sel4")
            nc.sync.dma_start(sel4[:], sel4_d[:])
            wom = []
            for b in range(B):
                t = pp.tile([128, E], F32, tag=f"wom{b}")
                nc.sync.dma_start(t[:], womask_d[b])
                wom.append(t)

            # ---- norm = cumsum(softplus(w)) ; layout s = 16p + f ----
            wsp = pp.tile([128, C], F32, tag="wsp")
            nc.scalar.activation(wsp[:], wrawT[:], AF.Exp)
            nc.scalar.activation(wsp[:], wsp[:], AF.Ln, bias=1.0)
            cum = [wsp]
            for i, k in enumerate((1, 2, 4, 8)):
                nxt = pp.tile([128, C], F32, tag=f"cum{i}")
                prev = cum[-1]
                pv = prev[:].rearrange("p (g f) -> p g f", g=HPC)
                nv = nxt[:].rearrange("p (g f) -> p g f", g=HPC)
                nc.vector.tensor_add(
                    nv[:, :, k:16], pv[:, :, k:16], pv[:, :, 0 : 16 - k]
                )
                nc.vector.tensor_copy(nv[:, :, 0:k], pv[:, :, 0:k])
                cum.append(nxt)
            cfin = cum[-1]
            # chunk totals (p, g) at f=15, exclusive prefix over partitions via PE
            t128 = cfin[:].rearrange("p (g f) -> p g f", g=HPC)[:, :, 15]
            pe_e = pse.tile([128, HPC], F32)
            nc.tensor.matmul(pe_e[:], lts[:], t128, start=True, stop=True)
            norm = pp.tile([128, C], F32, tag="norm")
            nc.vector.tensor_add(
                norm[:].rearrange("p (g f) -> p g f", g=HPC),
                cfin[:].rearrange("p (g f) -> p g f", g=HPC),
                pe_e[:, :, None].broadcast_to([128, HPC, 16]),
            )
            rnorm = pp.tile([128, C], F32, tag="rnorm")
            nc.vector.reciprocal(rnorm[:], norm[:])
            # rows layout (g, s) with s = 16p + f
            rnr = pp.tile([HPC, S], F32, tag="rnr")
            nrr = pp.tile([HPC, S], F32, tag="nrr")
            for g in range(HPC):
                nc.sync.dma_start(
                    rnr[g : g + 1, :], rnorm[:, g * 16 : (g + 1) * 16]
                )
                nc.sync.dma_start(
                    nrr[g : g + 1, :], norm[:, g * 16 : (g + 1) * 16]
                )
            bnorm = pp.tile([HPC, S], F32, tag="bnorm")
            nc.vector.tensor_mul(bnorm[:], biasr[:], nrr[:])

            # ---- shifted Toeplitz rows: softplus per head ----
            mstar = []
            for g in range(HPC):
                mt = pp.tile([128, JCOLS], F32, tag=f"mstar{g}", name=f"mstar{g}")
                nc.gpsimd.dma_start(mt[:], wsh_d[g])
                nc.scalar.activation(mt[:], mt[:], AF.Exp)
                mstar.append(mt)
            for g in range(HPC):
                nc.scalar.activation(mstar[g][:], mstar[g][:], AF.Ln, bias=1.0)

            # ---- input projection: psum_h[(s),(c)] per (st, b) ----
            h_sb = [pp.tile([128, 128], F32, tag=f"h{i}", name=f"h{i}") for i in range(NST)]
            for st in range(NST):
                xm = wstage.tile([128, B, E], F32, tag="xm", bufs=4)
                nc.sync.dma_start(
                    xm[:], xT_d[:, st * 2 : st * 2 + 2, :]
                )
                for b in range(B):
                    ph = psh.tile([128, C], F32)
                    for k in range(4):
                        nc.tensor.matmul(
                            ph[:],
                            xm[:, b, k * 128 : (k + 1) * 128],
                            wt[:, k, :],
                            start=(k == 0),
                            stop=(k == 3),
                        )
                    # scatter (g,d) -> columns 32g+16b+d, add inp_b
                    hv = h_sb[st][:].rearrange(
                        "p (g two d) -> p g two d", g=HPC, two=2
                    )[:, :, b, :]
                    nc.vector.tensor_add(
                        hv,
                        ph[:].rearrange("p (g d) -> p g d", g=HPC),
                        inpb[:].rearrange("p (g d) -> p g d", g=HPC),
                    )

            # ---- mixing: col-tiled 4 heads, accumulate over s-tiles ----
            y_sb = [pp.tile([128, 512], F32, tag=f"y{j}", name=f"y{j}") for j in range(NT)]
            for tj in range(NT):
                py = psy.tile([128, 512], F32)
                nsi = 4 * tj + 4
                for si in range(nsi):
                    j0 = 512 * tj - 128 * si + 384
                    for g in range(HPC):
                        nc.tensor.matmul(
                            py[32 * g : 32 * g + 32, :],
                            h_sb[si][:, 32 * g : 32 * g + 32],
                            mstar[g][:, j0 : j0 + 512],
                            start=(si == 0),
                            stop=False,
                            tile_position=(0, 32 * g),
                            skip_group_check=True,
                        )
                # + bias*norm broadcast to row groups (K=4 selector matmul)
                nc.tensor.matmul(
                    py[:],
                    sel4[:],
                    bnorm[:, 512 * tj : 512 * (tj + 1)],
                    start=False,
                    stop=True,
                    skip_group_check=True,
                )
                # rnorm broadcast to row groups, then y = psum * rm
                rm = pse.tile([128, 512], F32, tag="rm", name="rm", bufs=1)
                nc.tensor.matmul(
                    rm[:],
                    sel4[:],
                    rnr[:, 512 * tj : 512 * (tj + 1)],
                    start=True,
                    stop=True,
                )
                rms = wstage.tile([128, 512], F32, tag="rms", bufs=2)
                nc.scalar.activation(rms[:], rm[:], AF.Copy)
                nc.vector.tensor_mul(y_sb[tj][:], py[:], rms[:])

            # ---- output projection (masked per b), DMA psum -> DRAM ----
            for b in range(B):
                for tt in range(NST):
                    tj, cc = tt // 4, (tt % 4) * 128
                    po = pso.tile([128, E], F32)
                    nc.tensor.matmul(
                        po[:],
                        y_sb[tj][:, cc : cc + 128],
                        wom[b][:],
                        start=True,
                        stop=True,
                    )
                    ob = wstage.tile([128, E], F32, tag="osb", bufs=3)
                    nc.vector.tensor_copy(ob[:], po[:])
                    nc.scalar.dma_start(
                        out_d[b, tt * 128 : (tt + 1) * 128, :], ob[:]
                    )
    nc.compile()
    return nc


def _host_prep(x, weight_raw, bias, inp_w, inp_b, out_w):
    x = np.asarray(x, np.float32)
    weight_raw = np.asarray(weight_raw, np.float32)
    bias = np.asarray(bias, np.float32)
    inp_w = np.asarray(inp_w, np.float32)
    inp_b = np.asarray(inp_b, np.float32)
    out_w = np.asarray(out_w, np.float32)

    xT = x.transpose(2, 0, 1).reshape(E, B * S)
    xT = xT.reshape(4, 128, B, NST, 128)
    # [p, m'=(st,b), k*128+f]
    xT = np.ascontiguousarray(
        xT.transpose(1, 3, 2, 0, 4).reshape(128, B * NST, E)
    )

    p = np.arange(128)[:, None]
    j = np.arange(JCOLS)[None, :]
    lag = j - 384 - p
    valid = (lag >= 0) & (lag < S)
    lagc = np.clip(lag, 0, S - 1)

    lts = np.tril(np.ones((128, 128), np.float32), -1).T.copy()
    # lts[q, p] = 1 if q < p  -> strictly upper in (q, p) = tril(-1).T
    sel4 = (np.arange(128)[None, :] // 32 == np.arange(HPC)[:, None]).astype(
        np.float32
    )

    in_maps = []
    for core in range(NCORE):
        c0 = core * C
        heads = slice(core * HPC, (core + 1) * HPC)
        wsh = np.where(
            valid[None], weight_raw[heads][:, lagc], np.float32(PAD)
        ).astype(np.float32)
        wrawT = np.ascontiguousarray(
            weight_raw[heads].reshape(HPC, 128, 16).transpose(1, 0, 2).reshape(128, C)
        )
        # wrawT[p, 16g+f] = weight_raw[head g][16p+f]
        womask = np.zeros((B, 128, E), np.float32)
        wo_slice = out_w[:, c0 : c0 + C].T.astype(np.float32)  # (C=g*16+d, E)
        for b in range(B):
            v = womask[b].reshape(HPC, 2, D, E)
            v[:, b, :, :] = wo_slice.reshape(HPC, D, E)
        in_maps.append(
            {
                "xT": xT,
                "inpwT": np.ascontiguousarray(inp_w[c0 : c0 + C, :].T),
                "inpb": np.ascontiguousarray(np.broadcast_to(inp_b[c0 : c0 + C], (128, C))),
                "wsh": wsh,
                "wrawT": wrawT,
                "bias": bias[heads, :S].copy(),
                "womask": womask,
                "lts": lts,
                "sel4": sel4,
            }
        )
    return in_maps


def _run(in_maps, trace=False):
    if "nc" not in _CACHE:
        _CACHE["nc"] = _build_program()
    try:
        res = run_bass_kernel_spmd(
            _CACHE["nc"], in_maps, core_ids=list(range(NCORE)), trace=trace
        )
    except ModuleNotFoundError:
        res = run_bass_kernel_spmd(
            _CACHE["nc"], in_maps, core_ids=list(range(NCORE)), trace=False
        )
    return res


def kernel(x, weight_raw, bias, inp_w, inp_b, out_w, parallel=True, _trace=False):
    in_maps = _host_prep(x, weight_raw, bias, inp_w, inp_b, out_w)
    res = _run(in_maps, trace=_trace)
    out = np.zeros((B, S, E), np.float32)
    for r in res.results:
        out += r["out"]
    if _trace:
        kernel.last_exec_ns = res.exec_time_ns
        kernel.last_results = res
    return out


if __name__ == "__main__":
    rng = np.random.default_rng(0)
    inputs = {
        "x": rng.standard_normal((B, S, E), dtype=np.float32),
        "weight_raw": rng.standard_normal((H, S), dtype=np.float32),
        "bias": np.zeros((H, S), np.float32),
        "inp_w": rng.standard_normal((E, E), dtype=np.float32) / np.sqrt(E),
        "inp_b": np.zeros((E,), np.float32),
        "out_w": rng.standard_normal((E, E), dtype=np.float32) / np.sqrt(E),
    }
    o = kernel(**inputs)
    print("ok", o.shape, float(np.abs(o).mean()))


# revision 14
# speedup vs baseline: 1.6064x; 1.2435x over previous
import sys

import numpy as np

for _p in ("/opt/trn_rl_repo",):
    if _p not in sys.path:
        sys.path.insert(0, _p)

import concourse.bass as bass
import concourse.bacc as bacc
import concourse.mybir as mybir
import concourse.tile as tile
from concourse.bass_utils import run_bass_kernel_spmd

F32 = mybir.dt.float32
AF = mybir.ActivationFunctionType

# Problem dims (hardcoded per contract)
B, S, E, H = 2, 2048, 512, 32
D = E // H            # 16
NCORE = 8
HPC = H // NCORE      # 4 heads per core
C = HPC * D           # 64 channels per core
JCOLS = 2432          # shifted-weight row length: lag = j - 384 - p in [0, 2047]
PAD = -200.0          # softplus(PAD) == 0.0 in fp32
NT = S // 512         # 4 t-blocks of 512
NST = S // 128        # 16 s-tiles of 128

_CACHE = {}


def _build_program():
    nc = bacc.Bacc()

    xT_d = nc.dram_tensor("xT", [E, B * S], F32, kind="ExternalInput")
    inpwT_d = nc.dram_tensor("inpwT", [E, C], F32, kind="ExternalInput")
    inpb_d = nc.dram_tensor("inpb", [C, 1], F32, kind="ExternalInput")
    id64_d = nc.dram_tensor("id64", [64, 64], F32, kind="ExternalInput")
    wsh_d = nc.dram_tensor("wsh", [HPC, 128, JCOLS], F32, kind="ExternalInput")
    wrawT_d = nc.dram_tensor("wrawT", [128, C], F32, kind="ExternalInput")
    bias_d = nc.dram_tensor("bias", [HPC, S], F32, kind="ExternalInput")
    womask_d = nc.dram_tensor("womask", [B, 128, E], F32, kind="ExternalInput")
    lts_d = nc.dram_tensor("lts", [128, 128], F32, kind="ExternalInput")
    sel4_d = nc.dram_tensor("sel4", [HPC, 128], F32, kind="ExternalInput")
    out_d = nc.dram_tensor("out", [B, S, E], F32, kind="ExternalOutput")

    with tile.TileContext(nc) as tc:
        with (
            tc.tile_pool(name="persist", bufs=1) as pp,
            tc.tile_pool(name="wstage", bufs=2) as wstage,
            tc.tile_pool(name="psh", bufs=2, space="PSUM") as psh,
            tc.tile_pool(name="pstp", bufs=2, space="PSUM") as pst_pool,
            tc.tile_pool(name="psy", bufs=1, space="PSUM") as psy,
            tc.tile_pool(name="pso", bufs=2, space="PSUM") as pso,
            tc.tile_pool(name="pse", bufs=1, space="PSUM") as pse,
        ):
            # ---- small/constant loads ----
            wt = pp.tile([128, 4, C], F32, tag="wt")  # inp_wT as (p, k, c)
            nc.sync.dma_start(
                wt[:], inpwT_d[:].rearrange("(k p) c -> p k c", p=128)
            )
            inpb = pp.tile([C, 1], F32, tag="inpb")
            nc.sync.dma_start(inpb[:], inpb_d[:])
            id64 = pp.tile([64, 64], F32, tag="id64")
            nc.sync.dma_start(id64[:], id64_d[:])
            wrawT = pp.tile([128, C], F32, tag="wrawT")
            nc.sync.dma_start(wrawT[:], wrawT_d[:])
            biasr = pp.tile([HPC, S], F32, tag="biasr")
            nc.sync.dma_start(biasr[:], bias_d[:])
            lts = pp.tile([128, 128], F32, tag="lts")
            nc.sync.dma_start(lts[:], lts_d[:])
            sel4 = pp.tile([HPC, 128], F32, tag="sel4")
            nc.sync.dma_start(sel4[:], sel4_d[:])
            wom = []
            for b in range(B):
                t = pp.tile([128, E], F32, tag=f"wom{b}")
                nc.sync.dma_start(t[:], womask_d[b])
                wom.append(t)

            # ---- norm = cumsum(softplus(w)) ; layout s = 16p + f ----
            wsp = pp.tile([128, C], F32, tag="wsp")
            nc.scalar.activation(wsp[:], wrawT[:], AF.Exp)
            nc.scalar.activation(wsp[:], wsp[:], AF.Ln, bias=1.0)
            cum = [wsp]
            for i, k in enumerate((1, 2, 4, 8)):
                nxt = pp.tile([128, C], F32, tag=f"cum{i}")
                prev = cum[-1]
                pv = prev[:].rearrange("p (g f) -> p g f", g=HPC)
                nv = nxt[:].rearrange("p (g f) -> p g f", g=HPC)
                nc.vector.tensor_add(
                    nv[:, :, k:16], pv[:, :, k:16], pv[:, :, 0 : 16 - k]
                )
                nc.vector.tensor_copy(nv[:, :, 0:k], pv[:, :, 0:k])
                cum.append(nxt)
            cfin = cum[-1]
            # chunk totals (p, g) at f=15, exclusive prefix over partitions via PE
            t128 = cfin[:].rearrange("p (g f) -> p g f", g=HPC)[:, :, 15]
            pe_e = pse.tile([128, 512], F32, tag="rmtag", name="pe_e", bufs=1)
            nc.tensor.matmul(pe_e[:, 0:HPC], lts[:], t128, start=True, stop=True)
            norm = pp.tile([128, C], F32, tag="norm")
            nc.vector.tensor_add(
                norm[:].rearrange("p (g f) -> p g f", g=HPC),
                cfin[:].rearrange("p (g f) -> p g f", g=HPC),
                pe_e[:, 0:HPC, None].broadcast_to([128, HPC, 16]),
            )
            rnorm = pp.tile([128, C], F32, tag="rnorm")
            nc.vector.reciprocal(rnorm[:], norm[:])
            # rows layout (g, s) with s = 16p + f
            rnr = pp.tile([HPC, S], F32, tag="rnr")
            nrr = pp.tile([HPC, S], F32, tag="nrr")
            for g in range(HPC):
                nc.sync.dma_start(
                    rnr[g : g + 1, :], rnorm[:, g * 16 : (g + 1) * 16]
                )
                nc.sync.dma_start(
                    nrr[g : g + 1, :], norm[:, g * 16 : (g + 1) * 16]
                )
            bnorm = pp.tile([HPC, S], F32, tag="bnorm")
            nc.vector.tensor_mul(bnorm[:], biasr[:], nrr[:])

            # ---- shifted Toeplitz rows: softplus per head ----
            mstar = []
            for g in range(HPC):
                mt = pp.tile([128, JCOLS], F32, tag=f"mstar{g}", name=f"mstar{g}")
                nc.gpsimd.dma_start(mt[:], wsh_d[g])
                nc.scalar.activation(mt[:], mt[:], AF.Exp)
                mstar.append(mt)
            for g in range(HPC):
                nc.scalar.activation(mstar[g][:], mstar[g][:], AF.Ln, bias=1.0)

            # ---- x tiles (e-major, 4 resident k-slabs) ----
            xk = []
            for k in range(4):
                t = pp.tile([128, B * S], F32, tag=f"xk{k}", name=f"xk{k}")
                nc.sync.dma_start(t[:], xT_d[k * 128 : (k + 1) * 128, :])
                xk.append(t)

            # ---- input projection (M=c): hc[(c),(bs)] then PE-transpose ----
            h_sb = [pp.tile([128, 128], F32, tag=f"h{i}", name=f"h{i}") for i in range(NST)]
            for n in range(8):          # bs 512-blocks; n = b*4 + q
                b, q = n // 4, n % 4
                hc = psh.tile([64, 512], F32, name="hc")
                for k in range(4):
                    nc.tensor.matmul(
                        hc[:],
                        wt[:, k, :],
                        xk[k][:, n * 512 : (n + 1) * 512],
                        start=(k == 0),
                        stop=(k == 3),
                    )
                hcs = wstage.tile([64, 512], F32, tag="hcs", bufs=3)
                nc.vector.tensor_add(
                    hcs[:], hc[:], inpb[:, 0:1].broadcast_to([C, 512])
                )
                for tt in range(4):
                    st = q * 4 + tt
                    pst = pst_pool.tile([128, 64], F32, name="pst")
                    nc.tensor.transpose(
                        pst[:], hcs[:, tt * 128 : (tt + 1) * 128], id64[:]
                    )
                    hv = h_sb[st][:].rearrange(
                        "p (g two d) -> p g two d", g=HPC, two=2
                    )[:, :, b, :]
                    nc.vector.tensor_copy(
                        hv, pst[:].rearrange("p (g d) -> p g d", g=HPC)
                    )

            # ---- mixing + norm/bias + out-projection, interleaved per tj ----
            y_sb = [pp.tile([128, 512], F32, tag=f"y{j}", name=f"y{j}") for j in range(NT)]
            for tj in range(NT):
                py = psy.tile([128, 512], F32, name="py")
                nsi = 4 * tj + 4
                for si in range(nsi):
                    j0 = 512 * tj - 128 * si + 384
                    for g in range(HPC):
                        nc.tensor.matmul(
                            py[32 * g : 32 * g + 32, :],
                            h_sb[si][:, 32 * g : 32 * g + 32],
                            mstar[g][:, j0 : j0 + 512],
                            start=(si == 0),
                            stop=False,
                            tile_position=(0, 32 * g),
                            skip_group_check=True,
                        )
                # + bias*norm broadcast to row groups (K=4 selector matmul)
                nc.tensor.matmul(
                    py[:],
                    sel4[:],
                    bnorm[:, 512 * tj : 512 * (tj + 1)],
                    start=False,
                    stop=True,
                    skip_group_check=True,
                )
                # rnorm broadcast to row groups, then y = psum * rm
                rm = pse.tile([128, 512], F32, tag="rmtag", name="rm", bufs=1)
                nc.tensor.matmul(
                    rm[:],
                    sel4[:],
                    rnr[:, 512 * tj : 512 * (tj + 1)],
                    start=True,
                    stop=True,
                )
                rms = wstage.tile([128, 512], F32, tag="rms", bufs=2)
                nc.vector.tensor_copy(rms[:], rm[:])
                nc.vector.tensor_mul(y_sb[tj][:], py[:], rms[:])

                # out-projection for this tj (all 4 t-128-tiles, both b)
                for tt4 in range(4):
                    tt = tj * 4 + tt4
                    for b in range(B):
                        po = pso.tile([128, E], F32, name="po")
                        nc.tensor.matmul(
                            po[:],
                            y_sb[tj][:, tt4 * 128 : tt4 * 128 + 128],
                            wom[b][:],
                            start=True,
                            stop=True,
                        )
                        ob = wstage.tile([128, E], F32, tag="osb", bufs=3)
                        nc.vector.tensor_copy(ob[:], po[:])
                        nc.scalar.dma_start(
                            out_d[b, tt * 128 : (tt + 1) * 128, :], ob[:]
                        )
    nc.compile()
    return nc


def _host_prep(x, weight_raw, bias, inp_w, inp_b, out_w):
    x = np.asarray(x, np.float32)
    weight_raw = np.asarray(weight_raw, np.float32)
    bias = np.asarray(bias, np.float32)
    inp_w = np.asarray(inp_w, np.float32)
    inp_b = np.asarray(inp_b, np.float32)
    out_w = np.asarray(out_w, np.float32)

    xT = np.ascontiguousarray(x.transpose(2, 0, 1).reshape(E, B * S))

    p = np.arange(128)[:, None]
    j = np.arange(JCOLS)[None, :]
    lag = j - 384 - p
    valid = (lag >= 0) & (lag < S)
    lagc = np.clip(lag, 0, S - 1)

    lts = np.tril(np.ones((128, 128), np.float32), -1).T.copy()
    # lts[q, p] = 1 if q < p  -> strictly upper in (q, p) = tril(-1).T
    sel4 = (np.arange(128)[None, :] // 32 == np.arange(HPC)[:, None]).astype(
        np.float32
    )

    in_maps = []
    for core in range(NCORE):
        c0 = core * C
        heads = slice(core * HPC, (core + 1) * HPC)
        wsh = np.where(
            valid[None], weight_raw[heads][:, lagc], np.float32(PAD)
        ).astype(np.float32)
        wrawT = np.ascontiguousarray(
            weight_raw[heads].reshape(HPC, 128, 16).transpose(1, 0, 2).reshape(128, C)
        )
        # wrawT[p, 16g+f] = weight_raw[head g][16p+f]
        womask = np.zeros((B, 128, E), np.float32)
        wo_slice = out_w[:, c0 : c0 + C].T.astype(np.float32)  # (C=g*16+d, E)
        for b in range(B):
            v = womask[b].reshape(HPC, 2, D, E)
            v[:, b, :, :] = wo_slice.reshape(HPC, D, E)
        in_maps.append(
            {
                "xT": xT,
                "inpwT": np.ascontiguousarray(inp_w[c0 : c0 + C, :].T),
                "inpb": inp_b[c0 : c0 + C].reshape(C, 1).copy(),
                "id64": np.eye(64, dtype=np.float32),
                "wsh": wsh,
                "wrawT": wrawT,
                "bias": bias[heads, :S].copy(),
                "womask": womask,
                "lts": lts,
                "sel4": sel4,
            }
        )
    return in_maps


def _run(in_maps, trace=False):
    if "nc" not in _CACHE:
        _CACHE["nc"] = _build_program()
    try:
        res = run_bass_kernel_spmd(
            _CACHE["nc"], in_maps, core_ids=list(range(NCORE)), trace=trace
        )
    except ModuleNotFoundError:
        res = run_bass_kernel_spmd(
            _CACHE["nc"], in_maps, core_ids=list(range(NCORE)), trace=False
        )
    return res


def kernel(x, weight_raw, bias, inp_w, inp_b, out_w, parallel=True, _trace=False):
    in_maps = _host_prep(x, weight_raw, bias, inp_w, inp_b, out_w)
    res = _run(in_maps, trace=_trace)
    out = np.zeros((B, S, E), np.float32)
    for r in res.results:
        out += r["out"]
    if _trace:
        kernel.last_exec_ns = res.exec_time_ns
        kernel.last_results = res
    return out


if __name__ == "__main__":
    rng = np.random.default_rng(0)
    inputs = {
        "x": rng.standard_normal((B, S, E), dtype=np.float32),
        "weight_raw": rng.standard_normal((H, S), dtype=np.float32),
        "bias": np.zeros((H, S), np.float32),
        "inp_w": rng.standard_normal((E, E), dtype=np.float32) / np.sqrt(E),
        "inp_b": np.zeros((E,), np.float32),
        "out_w": rng.standard_normal((E, E), dtype=np.float32) / np.sqrt(E),
    }
    o = kernel(**inputs)
    print("ok", o.shape, float(np.abs(o).mean()))
